# revision 18
# baseline (speedup 1.0000x reference)
"""Trainium2 Bass kernel for nn_LongTermMemory (distributed brute-force kNN).

Strategy (classic distributed ANN pattern, per the sharding hint):
  - Host computes the tiny query projection q = Wq @ mean(query) + bq and
    pre-scales it by 1/sqrt(d) (exact power-of-two scale, bitwise-neutral).
  - The three key stores (1.6M x 256 fp32 = 1.64 GB) are sharded row-wise
    across 8 NeuronCores. Each core streams its 200k-key shard from HBM and
    computes all dot-product scores on-device -- the memory-roofline part.
  - The tiny remainder (top-k select over returned scores, value gather,
    rerank over 11 candidates) runs on host.

Per-core device work is split across two engines so HBM DMA is the
bottleneck (fp32 matmul streams at 1/4 rate, so PE alone is too slow):
  - PE part: keys pre-transposed on host (blocked [b, 2, 128, FB]), q
    stationary, scores accumulated over two 128-dim chunks into PSUM,
    evacuated by ScalarE in [1, 1024] slices.
  - DVE part: natural-layout keys, q broadcast along free dim,
    tensor_mul + segmented reduce_sum.
"""
import os
import sys

import numpy as np

for _p in ("/opt/trn_rl_repo", "/root/.axon_site/_ro/trn_rl_repo"):
    if os.path.isdir(_p) and _p not in sys.path:
        sys.path.append(_p)

P = 128          # SBUF partitions
D = 256          # key dim
NC = 8           # cores
FB = 2048        # PE: keys per block (block tile = [128, 2*FB] = 2 MiB)
MM = 512         # PE: keys per matmul (one PSUM bank of fp32)
HB = 1024        # PE: keys per PSUM tile ([1, HB] = 2 banks)
G = 16           # DVE: keys per partition per tile (tile = [128, G*D] = 2 MiB)

N_FACT, N_PROC, N_EPIS = 1_000_000, 100_000, 500_000
S_FACT, S_PROC, S_EPIS = N_FACT // NC, N_PROC // NC, N_EPIS // NC
N_C = S_FACT + S_PROC + S_EPIS          # 200000 keys per core

B_PE = 43                               # PE blocks
NPE = B_PE * FB                         # keys on the PE path
assert NPE <= S_FACT
T_DVE = 55                              # DVE tiles
NDVE = T_DVE * P * G                    # keys on the DVE path
NPAD = NPE + NDVE                       # 200704
assert NPAD >= N_C

_CACHE = {}
LAST_RESULTS = None  # BassKernelResults of the most recent run (for profiling)


def _build():
    if "nc" in _CACHE:
        return _CACHE["nc"]
    import concourse.bass as bass
    import concourse.bacc as bacc
    import concourse.tile as tile
    import concourse.mybir as mybir

    nc = bacc.Bacc("TRN2", target_bir_lowering=False, debug=False)
    ktp_d = nc.dram_tensor("keysT", [B_PE, 2, P, FB], mybir.dt.float32,
                           kind="ExternalInput")
    kn_d = nc.dram_tensor("keysN", [NDVE, D], mybir.dt.float32,
                          kind="ExternalInput")
    q_d = nc.dram_tensor("q2", [P, 2], mybir.dt.float32, kind="ExternalInput")
    qr_d = nc.dram_tensor("qrep", [P, D], mybir.dt.float32, kind="ExternalInput")
    sp_d = nc.dram_tensor("scores_pe", [B_PE, FB], mybir.dt.float32,
                          kind="ExternalOutput")
    sd_d = nc.dram_tensor("scores_dve", [P, T_DVE * G], mybir.dt.float32,
                          kind="ExternalOutput")

    # one 2 MiB DMA per PE block: partition p <- [chunk0 row p | chunk1 row p]
    ktp_r = ktp_d.ap().rearrange("b c p f -> b p c f")
    kn_r = kn_d.ap().rearrange("(t p g) d -> t p (g d)", p=P, g=G)

    with tile.TileContext(nc) as tc:
        with tc.tile_pool(name="kpe", bufs=4) as kpool, \
             tc.tile_pool(name="kdve", bufs=4) as npool, \
             tc.tile_pool(name="scout", bufs=1) as spool, \
             tc.tile_pool(name="sdve", bufs=1) as sdpool, \
             tc.tile_pool(name="qp", bufs=1) as qpool, \
             tc.tile_pool(name="ps", bufs=4, space=bass.MemorySpace.PSUM) as pspool:
            qt = qpool.tile([P, 2], mybir.dt.float32)
            nc.gpsimd.dma_start(qt[:], q_d.ap())
            qr = qpool.tile([P, D], mybir.dt.float32)
            nc.gpsimd.dma_start(qr[:], qr_d.ap())
            qb = qr[:].rearrange("p d -> p () d").broadcast_to([P, G, D])
            sd = sdpool.tile([P, T_DVE * G], mybir.dt.float32)

            # scores_pe viewed flat for batched out-DMA
            sp_flat = sp_d.ap().rearrange("b f -> () (b f)")
            OUTW = 8192          # scout batch: 8 half-blocks = 32 KiB
            scout_state = {"tile": None, "filled": 0, "base": 0}

            def flush_scout():
                st = scout_state
                if st["tile"] is not None and st["filled"] > 0:
                    w = st["filled"] * HB
                    nc.gpsimd.dma_start(
                        sp_flat[0:1, st["base"]:st["base"] + w],
                        st["tile"][0:1, :w])
                    st["tile"] = None
                    st["filled"] = 0

            def pe_block(b):
                kt = kpool.tile([P, 2 * FB], mybir.dt.float32)
                nc.sync.dma_start(
                    kt[:].rearrange("p (c f) -> p c f", c=2), ktp_r[b])
                for h in range(FB // HB):
                    ps = pspool.tile([1, HB], mybir.dt.float32)
                    for g in range(HB // MM):
                        o = h * HB + g * MM
                        for c in range(2):
                            nc.tensor.matmul(
                                ps[0:1, g * MM:(g + 1) * MM],
                                qt[:, c:c + 1],
                                kt[:, c * FB + o:c * FB + o + MM],
                                start=(c == 0),
                                stop=(c == 1),
                            )
                    st = scout_state
                    if st["tile"] is None:
                        st["tile"] = spool.tile([1, OUTW], mybir.dt.float32,
                                                name="scout_batch")
                        st["base"] = (b * FB + h * HB)
                    nc.scalar.copy(
                        st["tile"][0:1, st["filled"] * HB:(st["filled"] + 1) * HB],
                        ps[0:1, :])
                    st["filled"] += 1
                    if st["filled"] * HB == OUTW:
                        flush_scout()

            def dve_tile(t):
                kt = npool.tile([P, G * D], mybir.dt.float32)
                nc.scalar.dma_start(kt[:], kn_r[t])
                k3 = kt[:].rearrange("p (g d) -> p g d", g=G)
                nc.vector.tensor_mul(k3, k3, qb)   # in-place k *= q
                nc.vector.reduce_sum(sd[:, t * G:(t + 1) * G], k3,
                                     axis=mybir.AxisListType.X)
                if t % 8 == 7:
                    nc.gpsimd.dma_start(sd_d.ap()[:, (t - 7) * G:(t + 1) * G],
                                        sd[:, (t - 7) * G:(t + 1) * G])

            # interleave so DMA demand and engine work stay even
            nb = max(B_PE, T_DVE)
            for i in range(nb):
                if i < B_PE:
                    pe_block(i)
                if i < T_DVE:
                    dve_tile(i)
            flush_scout()
            rem = (T_DVE // 8) * 8
            if rem < T_DVE:
                nc.gpsimd.dma_start(sd_d.ap()[:, rem * G:],
                                    sd[:, rem * G:])
    nc.compile()
    _CACHE["nc"] = nc
    return nc


def _make_in_map(fact, proc, epis, q2, qrep, core):
    f = fact[core * S_FACT:(core + 1) * S_FACT]
    # blocked transpose: [B_PE, 2, 128, FB], block b holds keys [b*FB,(b+1)*FB)
    kt_pe = np.ascontiguousarray(f[:NPE].T).reshape(2, P, B_PE, FB)
    kt_pe = np.ascontiguousarray(kt_pe.transpose(2, 0, 1, 3))
    kn = np.zeros((NDVE, D), dtype=np.float32)
    n_tail = S_FACT - NPE
    kn[:n_tail] = f[NPE:]
    o = n_tail
    kn[o:o + S_PROC] = proc[core * S_PROC:(core + 1) * S_PROC]
    o += S_PROC
    kn[o:o + S_EPIS] = epis[core * S_EPIS:(core + 1) * S_EPIS]
    return {"keysT": kt_pe, "keysN": kn, "q2": q2, "qrep": qrep}


def _topk_desc(scores, k):
    """jax.lax.top_k semantics: values descending, ties -> lower index."""
    k = int(k)
    if k * 8 < scores.shape[0]:
        cand = np.argpartition(-scores, k)[:k]
    else:
        cand = np.arange(scores.shape[0])
    order = np.lexsort((cand, -scores[cand]))[:k]
    return cand[order]


def kernel(query, factual_keys, factual_values, procedural_keys,
           procedural_values, episodic_keys, episodic_values,
           Wq, bq, Wr, br, top_k):
    global LAST_RESULTS
    query = np.asarray(query, dtype=np.float32)
    fact_k = np.asarray(factual_keys, dtype=np.float32)
    proc_k = np.asarray(procedural_keys, dtype=np.float32)
    epis_k = np.asarray(episodic_keys, dtype=np.float32)
    Wq = np.asarray(Wq, dtype=np.float32)
    bq = np.asarray(bq, dtype=np.float32)
    Wr = np.asarray(Wr, dtype=np.float32)
    br = np.float32(np.asarray(br))
    k = int(top_k)
    k_small = min(k, 3)

    # host: query projection (tiny)
    qm = query.mean(axis=0, dtype=np.float32)
    q = (Wq @ qm + bq).astype(np.float32)
    q_scaled = (q * np.float32(1.0 / 16.0)).astype(np.float32)  # 1/sqrt(256)
    q2 = np.ascontiguousarray(q_scaled.reshape(2, P).T)          # [128, 2]
    qrep = np.ascontiguousarray(np.broadcast_to(q_scaled, (P, D)))

    nc = _build()
    in_maps = [
        _make_in_map(fact_k, proc_k, epis_k, q2, qrep, c) for c in range(NC)
    ]
    from concourse.bass_utils import run_bass_kernel_spmd
    res = run_bass_kernel_spmd(nc, in_maps, core_ids=list(range(NC)))
    LAST_RESULTS = res

    per_core = []
    for c in range(NC):
        s_pe = res.results[c]["scores_pe"].reshape(-1)
        s_dve = (res.results[c]["scores_dve"]
                 .reshape(P, T_DVE, G).transpose(1, 0, 2).reshape(-1))
        per_core.append(np.concatenate([s_pe, s_dve])[:N_C])
    f_sc = np.concatenate([s[:S_FACT] for s in per_core])
    p_sc = np.concatenate([s[S_FACT:S_FACT + S_PROC] for s in per_core])
    e_sc = np.concatenate([s[S_FACT + S_PROC:] for s in per_core])

    # host: top-k per store, gather values, rerank (tiny)
    fi = _topk_desc(f_sc, k)
    pi = _topk_desc(p_sc, k_small)
    ei = _topk_desc(e_sc, k_small)
    vals = np.concatenate([
        np.asarray(factual_values)[fi],
        np.asarray(procedural_values)[pi],
        np.asarray(episodic_values)[ei],
    ]).astype(np.float32)
    svec = np.concatenate([f_sc[fi], p_sc[pi], e_sc[ei]]).astype(np.float32)

    n_cand = vals.shape[0]
    feats = np.concatenate(
        [np.broadcast_to(qm, (n_cand, D)), vals], axis=1).astype(np.float32)
    rscores = (feats @ Wr + br).astype(np.float32)
    idx = _topk_desc(rscores, k)
    return vals[idx], svec[idx]


# revision 19
# speedup vs baseline: 2.0469x; 2.0469x over previous
"""Trainium2 Bass kernel for nn_LongTermMemory (distributed brute-force kNN).

Strategy (classic distributed ANN pattern, per the sharding hint):
  - Host computes the tiny query projection q = Wq @ mean(query) + bq.
  - The three key stores (1.6M x 256 = 1.64 GB fp32) are sharded row-wise
    across 8 NeuronCores. Keys are rounded to bf16 on host (halving HBM
    traffic); each core streams its 200k-key shard and scores every key
    against q on the TensorEngine (bf16 matmul, fp32 PSUM accumulation).
  - Host takes a top-64 superset per store from the device scores, then
    re-scores those candidates in exact fp32. Since the bf16 score error
    (~7e-4) is far below the margin between the true top-k and the 64th
    candidate (~0.05+), the final top-k, svec and vals are exact fp32
    results -- identical (to ~1e-7) to scoring everything in fp32.
  - Value gather and the 11-candidate rerank run on host (tiny).

Device layout per core: keysT [98, 2, 128, 2048] bf16 -- per block b, the
2048 keys' 256 dims split into two 128-partition chunks (contraction dim on
partitions); q stationary [128, 2] bf16; scores [98, 2048] fp32 out in
natural key order. One 1 MiB DMA per block on the SP HWDGE ring (reads
only); score write-out batched on the gpsimd SWDGE ring; PSUM evacuated by
ScalarE.
"""
import os
import sys

import numpy as np

for _p in ("/opt/trn_rl_repo", "/root/.axon_site/_ro/trn_rl_repo"):
    if os.path.isdir(_p) and _p not in sys.path:
        sys.path.append(_p)

P = 128          # SBUF partitions
D = 256          # key dim
NC = 8           # cores
FB = 2048        # keys per block
MM = 512         # keys per matmul (one PSUM bank of fp32)

N_FACT, N_PROC, N_EPIS = 1_000_000, 100_000, 500_000
S_FACT, S_PROC, S_EPIS = N_FACT // NC, N_PROC // NC, N_EPIS // NC
N_C = S_FACT + S_PROC + S_EPIS          # 200000 keys per core
NBLK = -(-N_C // FB)                    # 98
NPAD = NBLK * FB                        # 200704

N_SUPER = 64     # per-store candidate superset re-scored exactly on host

_CACHE = {}
LAST_RESULTS = None  # BassKernelResults of the most recent run (for profiling)


def _build():
    if "nc" in _CACHE:
        return _CACHE["nc"]
    import concourse.bass as bass
    import concourse.bacc as bacc
    import concourse.tile as tile
    import concourse.mybir as mybir

    nc = bacc.Bacc("TRN2", target_bir_lowering=False, debug=False)
    kt_d = nc.dram_tensor("keysT", [NBLK, 2, P, FB], mybir.dt.bfloat16,
                          kind="ExternalInput")
    q_d = nc.dram_tensor("q2", [P, 2], mybir.dt.bfloat16, kind="ExternalInput")
    sc_d = nc.dram_tensor("scores", [NBLK, FB], mybir.dt.float32,
                          kind="ExternalOutput")

    kt_r = kt_d.ap().rearrange("b c p f -> b p c f")
    sc_flat = sc_d.ap().rearrange("b f -> () (b f)")

    with tile.TileContext(nc) as tc:
        with tc.tile_pool(name="kin", bufs=10) as kpool, \
             tc.tile_pool(name="scout", bufs=2) as spool, \
             tc.tile_pool(name="qp", bufs=1) as qpool, \
             tc.tile_pool(name="ps", bufs=2, space=bass.MemorySpace.PSUM) as pspool:
            qt = qpool.tile([P, 2], mybir.dt.bfloat16)
            nc.gpsimd.dma_start(qt[:], q_d.ap())

            OUTB = 4             # blocks per scout batch (32 KiB write)
            scout_state = {"tile": None, "filled": 0, "base": 0}

            def flush_scout():
                st = scout_state
                if st["tile"] is not None and st["filled"] > 0:
                    w = st["filled"] * FB
                    nc.gpsimd.dma_start(
                        sc_flat[0:1, st["base"]:st["base"] + w],
                        st["tile"][0:1, :w])
                    st["tile"] = None
                    st["filled"] = 0

            for b in range(NBLK):
                kt = kpool.tile([P, 2 * FB], mybir.dt.bfloat16)
                nc.sync.dma_start(
                    kt[:].rearrange("p (c f) -> p c f", c=2), kt_r[b])
                ps = pspool.tile([1, FB], mybir.dt.float32)
                for g in range(FB // MM):
                    for c in range(2):
                        nc.tensor.matmul(
                            ps[0:1, g * MM:(g + 1) * MM],
                            qt[:, c:c + 1],
                            kt[:, c * FB + g * MM:c * FB + (g + 1) * MM],
                            start=(c == 0),
                            stop=(c == 1),
                        )
                st = scout_state
                if st["tile"] is None:
                    st["tile"] = spool.tile([1, OUTB * FB], mybir.dt.float32,
                                            name="scout_batch")
                    st["base"] = b * FB
                nc.scalar.copy(
                    st["tile"][0:1, st["filled"] * FB:(st["filled"] + 1) * FB],
                    ps[0:1, :])
                st["filled"] += 1
                if st["filled"] == OUTB:
                    flush_scout()
            flush_scout()
    nc.compile()
    _CACHE["nc"] = nc
    return nc


def _make_in_map(fact, proc, epis, q2, core):
    import ml_dtypes
    kc = np.zeros((D, NPAD), dtype=ml_dtypes.bfloat16)
    o = 0
    for arr, sz in ((fact, S_FACT), (proc, S_PROC), (epis, S_EPIS)):
        kc[:, o:o + sz] = arr[core * sz:(core + 1) * sz].astype(
            ml_dtypes.bfloat16).T
        o += sz
    kt = np.ascontiguousarray(
        kc.reshape(2, P, NBLK, FB).transpose(2, 0, 1, 3))
    return {"keysT": kt, "q2": q2}


def _topk_desc(scores, k):
    """jax.lax.top_k semantics: values descending, ties -> lower index."""
    k = int(k)
    if k * 8 < scores.shape[0]:
        cand = np.argpartition(-scores, k)[:k]
    else:
        cand = np.arange(scores.shape[0])
    order = np.lexsort((cand, -scores[cand]))[:k]
    return cand[order]


def kernel(query, factual_keys, factual_values, procedural_keys,
           procedural_values, episodic_keys, episodic_values,
           Wq, bq, Wr, br, top_k):
    global LAST_RESULTS
    import ml_dtypes
    query = np.asarray(query, dtype=np.float32)
    fact_k = np.asarray(factual_keys, dtype=np.float32)
    proc_k = np.asarray(procedural_keys, dtype=np.float32)
    epis_k = np.asarray(episodic_keys, dtype=np.float32)
    Wq = np.asarray(Wq, dtype=np.float32)
    bq = np.asarray(bq, dtype=np.float32)
    Wr = np.asarray(Wr, dtype=np.float32)
    br = np.float32(np.asarray(br))
    k = int(top_k)
    k_small = min(k, 3)

    # host: query projection (tiny); 1/sqrt(256) folded into q (exact pow2)
    qm = query.mean(axis=0, dtype=np.float32)
    q = (Wq @ qm + bq).astype(np.float32)
    q_scaled = (q * np.float32(1.0 / 16.0)).astype(np.float32)
    q2 = np.ascontiguousarray(
        q_scaled.reshape(2, P).T.astype(ml_dtypes.bfloat16))

    nc = _build()
    in_maps = [_make_in_map(fact_k, proc_k, epis_k, q2, c) for c in range(NC)]
    from concourse.bass_utils import run_bass_kernel_spmd
    res = run_bass_kernel_spmd(nc, in_maps, core_ids=list(range(NC)))
    LAST_RESULTS = res

    per_core = [res.results[c]["scores"].reshape(-1)[:N_C] for c in range(NC)]
    f_sc = np.concatenate([s[:S_FACT] for s in per_core])
    p_sc = np.concatenate([s[S_FACT:S_FACT + S_PROC] for s in per_core])
    e_sc = np.concatenate([s[S_FACT + S_PROC:] for s in per_core])

    def exact_topk(approx_scores, keys, kk):
        """top-kk by exact fp32 scores, selected from the approximate
        device top-N_SUPER superset (bf16 error << superset margin)."""
        n_sup = min(N_SUPER, approx_scores.shape[0])
        cand = np.argpartition(-approx_scores, n_sup - 1)[:n_sup]
        exact = (keys[cand] @ q_scaled).astype(np.float32)
        order = np.lexsort((cand, -exact))[:kk]
        return cand[order], exact[order]

    fi, fs = exact_topk(f_sc, fact_k, k)
    pi, ps_ = exact_topk(p_sc, proc_k, k_small)
    ei, es = exact_topk(e_sc, epis_k, k_small)

    vals = np.concatenate([
        np.asarray(factual_values)[fi],
        np.asarray(procedural_values)[pi],
        np.asarray(episodic_values)[ei],
    ]).astype(np.float32)
    svec = np.concatenate([fs, ps_, es]).astype(np.float32)

    n_cand = vals.shape[0]
    feats = np.concatenate(
        [np.broadcast_to(qm, (n_cand, D)), vals], axis=1).astype(np.float32)
    rscores = (feats @ Wr + br).astype(np.float32)
    idx = _topk_desc(rscores, k)
    return vals[idx], svec[idx]
